# revision 5
# baseline (speedup 1.0000x reference)
"""Trainium2 Bass/Tile kernel for ExtAttentionPool (nn_ExtAttentionPool).

Math (per sample b):
    S[u, o]  = sum_d L[u, d] * W[o, d]
    E[o, u]  = exp(S[u,o]/O + b[o]/O)          (softmax numerator over u)
    Z[o]     = sum_u E[o, u]
    OUT[o,t] = (1/Z[o]) * sum_c E[o, c] * L[t, c]
    result row b = OUT flattened (O-major), shape (O*T,)

Sharding: data-parallel over batch B=16 across 8 cores (2 samples/core).

v2 design (stream-first):
  - The gpsimd (SWDGE) queue carries ONLY the logits cast-DMAs (f32->bf16
    inline), emitted as its first instructions, so the 8 MiB HBM stream
    starts at ~1 us instead of ~10 us. W^T (pre-transposed on host),
    b/O and a 128x128 f32 identity arrive as extra DRAM inputs on the
    sync HWDGE queue; DVE casts identity/WT to bf16.
  - L is transposed on PE (regular bf16 matmul vs identity -> FWL,
    ~81 ns/block sustained); PSUM->SBUF copy slabs alternate DVE/ACT.
  - mm1 (scores) runs in wide pieces as chunks land; exp on ACT
    accumulates Z inline; E-transpose on PE is tiny (10-col).
  - mm2 pre-accumulates all c-steps whose EC block and LT columns are
    already resident, so after the last chunk only: its transposes,
    one 128-wide mm1 piece, exp, one E-transpose, and one final c-step
    per output range remain (~4 us tail).
"""

import numpy as np
from contextlib import ExitStack

import concourse.bass as bass
import concourse.mybir as mybir
import concourse.tile as tile
from concourse import bacc
from concourse.bass_utils import run_bass_kernel_spmd

F32 = mybir.dt.float32
BF16 = mybir.dt.bfloat16

N_CORES = 8
B_FULL = 16


def build_nc(b_per=2, T=1024, D=1024, O=10, warmup_mms=6):
    """Build the per-core Bass program (bf16 compute). Same on all 8 cores."""
    P = 128
    NT = T // P            # 128-row t-blocks
    ND = D // P            # 128-col d-blocks
    assert NT == 8 and ND == 8 and b_per == 2

    # chunk plans in 128-row units: (r0, rj). s0 starts with two 1-block
    # chunks so PE work begins ~2.3us in (keeps the HAM gate open); s1
    # ends with two 1-block chunks so the tail is short.
    plans = [
        [(0, 1), (1, 1), (2, 2), (4, 2), (6, 2)],
        [(0, 2), (2, 2), (4, 2), (6, 1), (7, 1)],
    ]
    # mm1 pieces per sample: (t_off, width)
    mm1_pieces = [
        [(0, 512), (512, 512)],
        [(0, 512), (512, 256), (768, 128), (896, 128)],
    ]

    nc = bacc.Bacc(
        "TRN2", target_bir_lowering=False, debug=False, enable_asserts=False
    )
    logits = nc.dram_tensor("logits", (b_per, T, D), F32, kind="ExternalInput").ap()
    wt_in = nc.dram_tensor("wt", (P, ND, O), F32, kind="ExternalInput").ap()
    bias_in = nc.dram_tensor("b_over_o", (O, 1), F32, kind="ExternalInput").ap()
    ident_in = nc.dram_tensor("ident", (P, P), F32, kind="ExternalInput").ap()
    out = nc.dram_tensor("out", (b_per, O * T), F32, kind="ExternalOutput").ap()

    n_chunks = sum(len(p) for p in plans)

    with tile.TileContext(nc) as tc, ExitStack() as ctx:
        singles = ctx.enter_context(tc.tile_pool(name="singles", bufs=1))
        lr_pool = ctx.enter_context(tc.tile_pool(name="lr", bufs=n_chunks))
        lt_pool = ctx.enter_context(tc.tile_pool(name="lt", bufs=2))
        e_pool = ctx.enter_context(tc.tile_pool(name="e", bufs=2))
        z_pool = ctx.enter_context(tc.tile_pool(name="z", bufs=2))
        osb_pool = ctx.enter_context(tc.tile_pool(name="osb", bufs=2))
        slab_ps = ctx.enter_context(tc.tile_pool(name="slab", bufs=3, space="PSUM"))
        s_ps = ctx.enter_context(tc.tile_pool(name="sps", bufs=2, space="PSUM"))
        o_ps = ctx.enter_context(tc.tile_pool(name="ops", bufs=2, space="PSUM"))
        et_ps = ctx.enter_context(tc.tile_pool(name="etps", bufs=1, space="PSUM"))

        # --- gpsimd: ONLY the logits cast-DMAs, all up front ---
        lr_tiles = {}
        for s in range(b_per):
            for ci, (r0, rj) in enumerate(plans[s]):
                lr = lr_pool.tile([P, 2, D], BF16, tag="lr", name=f"lr_s{s}c{ci}")
                nc.gpsimd.dma_start(
                    out=lr[:, :rj, :],
                    in_=logits[
                        s, r0 * P : (r0 + rj) * P, :
                    ].rearrange("(j p) d -> p j d", p=P),
                )
                lr_tiles[(s, ci)] = (lr, r0, rj)

        # --- sync HWDGE: small constant inputs ---
        identf = singles.tile([P, P], F32)
        nc.sync.dma_start(out=identf, in_=ident_in)
        wtf = singles.tile([P, ND, O], F32)
        nc.sync.dma_start(out=wtf, in_=wt_in)
        bias01 = singles.tile([O, 1], F32)
        nc.sync.dma_start(out=bias01, in_=bias_in)

        # DVE casts of the constants
        ident = singles.tile([P, P], BF16)
        nc.vector.tensor_copy(ident, identf)
        wt_sb = singles.tile([P, ND, O], BF16)
        nc.vector.tensor_copy(wt_sb, wtf)

        # --- PE warmup (HAM clock gate) ---
        warm = slab_ps.tile([P, 4 * P], F32, tag="slab")
        for i in range(warmup_mms):
            nc.tensor.matmul(
                warm[:, (i % 4) * P : (i % 4 + 1) * P],
                lhsT=identf, rhs=identf, start=True, stop=True,
            )

        i_copy = [0]

        def transpose_block(lr, lt, j, r, engines=None):
            """PE-transpose row-block r (slot j of lr) into lt; copies on
            the engines given (list of 2, one per 4-wide slab)."""
            for g in range(2):
                slab = slab_ps.tile([P, 4 * P], F32, tag="slab")
                for k in range(4):
                    c = 4 * g + k
                    nc.tensor.matmul(
                        slab[:, k * P : (k + 1) * P],
                        lhsT=lr[:, j, c * P : (c + 1) * P],
                        rhs=ident,
                        start=True, stop=True,
                    )
                dst = lt[:, 4 * g : 4 * g + 4, r * P : (r + 1) * P]
                if engines is not None:
                    eng = engines[g]
                else:
                    eng = "v" if i_copy[0] % 2 == 0 else "s"
                    i_copy[0] += 1
                if eng == "v":
                    nc.vector.tensor_copy(dst, slab)
                else:
                    nc.scalar.activation(
                        out=dst, in_=slab,
                        func=mybir.ActivationFunctionType.Copy,
                    )

        def mm1_piece(s, lt, e_sb, zparts, pi, off, w):
            sp = s_ps.tile([O, w], F32, tag="sps", name=f"sp{s}_{pi}")
            for c in range(ND):
                nc.tensor.matmul(
                    sp,
                    lhsT=wt_sb[:, c, :],
                    rhs=lt[:, c, off : off + w],
                    start=(c == 0),
                    stop=(c == ND - 1),
                )
            nc.scalar.activation(
                out=e_sb[:, off : off + w],
                in_=sp,
                func=mybir.ActivationFunctionType.Exp,
                scale=1.0 / O,
                bias=bias01,
                accum_out=zparts[:, pi : pi + 1],
            )

        def et_blocks(e_sb, et_stage, ec, tbs, eng="v"):
            """PE-transpose E time-blocks tbs into ec (bf16)."""
            for tb in tbs:
                nc.tensor.matmul(
                    et_stage[:, tb, :],
                    lhsT=e_sb[:, tb * P : (tb + 1) * P],
                    rhs=ident[:O, :O],
                    start=True, stop=True,
                )
            lo, hi = min(tbs), max(tbs) + 1
            if eng == "v":
                nc.vector.tensor_copy(ec[:, lo:hi, :], et_stage[:, lo:hi, :])
            else:
                nc.scalar.activation(
                    out=ec[:, lo:hi, :], in_=et_stage[:, lo:hi, :],
                    func=mybir.ActivationFunctionType.Copy,
                )

        def mm2_step(op, ec, lt, c, t0, t1, start, stop):
            nc.tensor.matmul(
                op[:, 0 : t1 - t0],
                lhsT=ec[:, c, :],
                rhs=lt[:, c, t0:t1],
                start=start,
                stop=stop,
            )

        # ================= sample 0 (no tail pressure) =================
        s = 0
        lt0 = lt_pool.tile([P, ND, T], BF16, tag="lt")
        e0 = e_pool.tile([O, T], BF16, tag="e")
        et_stage0 = et_ps.tile([P, ND, O], F32, tag="etps")
        ec0 = e_pool.tile([P, ND, O], BF16, tag="ec")
        z0 = z_pool.tile([O, len(mm1_pieces[0])], F32, tag="z")

        done_rows = 0
        p_done = 0
        for ci, (r0, rj) in enumerate(plans[s]):
            lr, _, _ = lr_tiles[(s, ci)]
            for j in range(rj):
                transpose_block(lr, lt0, j, r0 + j)
            done_rows = (r0 + rj) * P
            while p_done < len(mm1_pieces[0]) and done_rows >= sum(
                mm1_pieces[0][p_done]
            ):
                off, w = mm1_pieces[0][p_done]
                mm1_piece(s, lt0, e0, z0, p_done, off, w)
                et_blocks(e0, et_stage0, ec0, list(range(off // P, (off + w) // P)))
                p_done += 1

        # Z and mm2 for sample 0: emitted now; deps all land mid-stream.
        zsum0 = z_pool.tile([O, 1], F32, tag="zs")
        nc.vector.reduce_sum(zsum0, z0, axis=mybir.AxisListType.X)
        rz0 = z_pool.tile([O, 1], F32, tag="rz")
        nc.vector.reciprocal(rz0, zsum0)

        opA0 = o_ps.tile([O, 512], F32, tag="ops", name="opA0")
        opB0 = o_ps.tile([O, 512], F32, tag="ops", name="opB0")
        for c in range(ND):
            mm2_step(opA0, ec0, lt0, c, 0, 512, c == 0, c == ND - 1)
            mm2_step(opB0, ec0, lt0, c, 512, 1024, c == 0, c == ND - 1)

        out2d0 = out[0].rearrange("(o t) -> o t", o=O)
        o_sb0 = osb_pool.tile([O, T], F32, tag="osb")
        nc.scalar.activation(
            out=o_sb0[:, 0:512], in_=opA0,
            func=mybir.ActivationFunctionType.Copy, scale=rz0,
        )
        nc.sync.dma_start(out=out2d0[:, 0:512], in_=o_sb0[:, 0:512])
        nc.vector.tensor_scalar_mul(o_sb0[:, 512:1024], opB0, rz0)
        nc.scalar.dma_start(out=out2d0[:, 512:1024], in_=o_sb0[:, 512:1024])

        # ================= sample 1 (tail-optimized) =================
        s = 1
        lt1 = lt_pool.tile([P, ND, T], BF16, tag="lt")
        e1 = e_pool.tile([O, T], BF16, tag="e")
        et_stage1 = et_ps.tile([P, ND, O], F32, tag="etps")
        ec1 = e_pool.tile([P, ND, O], BF16, tag="ec")
        z1 = z_pool.tile([O, len(mm1_pieces[1])], F32, tag="z")

        opA1 = o_ps.tile([O, 512], F32, tag="ops", name="opA1")   # cols 0:512
        opB = o_ps.tile([O, 384], F32, tag="ops", name="opB1")    # cols 512:896

        # chunk 0: rows 0:256
        lr, _, _ = lr_tiles[(s, 0)]
        transpose_block(lr, lt1, 0, 0)
        transpose_block(lr, lt1, 1, 1)
        # chunk 1: rows 256:512
        lr, _, _ = lr_tiles[(s, 1)]
        transpose_block(lr, lt1, 0, 2)
        transpose_block(lr, lt1, 1, 3)
        # piece 0: t 0:512 -> E blocks 0..3 -> opA presteps c=0..3
        mm1_piece(s, lt1, e1, z1, 0, 0, 512)
        et_blocks(e1, et_stage1, ec1, [0, 1, 2, 3])
        for c in range(4):
            mm2_step(opA1, ec1, lt1, c, 0, 512, c == 0, False)
        # chunk 2: rows 512:768
        lr, _, _ = lr_tiles[(s, 2)]
        transpose_block(lr, lt1, 0, 4)
        transpose_block(lr, lt1, 1, 5)
        mm1_piece(s, lt1, e1, z1, 1, 512, 256)
        et_blocks(e1, et_stage1, ec1, [4, 5])
        for c in (4, 5):
            mm2_step(opA1, ec1, lt1, c, 0, 512, False, False)
        # chunk 3: rows 768:896
        lr, _, _ = lr_tiles[(s, 3)]
        transpose_block(lr, lt1, 0, 6)
        mm1_piece(s, lt1, e1, z1, 2, 768, 128)
        et_blocks(e1, et_stage1, ec1, [6])
        mm2_step(opA1, ec1, lt1, 6, 0, 512, False, False)
        # opB (cols 512:896) needs rows 512:896 -> ready now; c=0..6
        for c in range(7):
            mm2_step(opB, ec1, lt1, c, 512, 896, c == 0, False)

        # ---- tail: chunk 4, rows 896:1024 ----
        lr, _, _ = lr_tiles[(s, 4)]
        transpose_block(lr, lt1, 0, 7, engines=("v", "s"))
        # opC (cols 896:1024) lives in a free s_ps bank — its own
        # accumulation group; c=0..6 against already-resident EC blocks
        opC = s_ps.tile([O, 128], F32, tag="sps", name="opC")
        for c in range(7):
            mm2_step(opC, ec1, lt1, c, 896, 1024, c == 0, False)
        mm1_piece(s, lt1, e1, z1, 3, 896, 128)
        et_blocks(e1, et_stage1, ec1, [7], eng="v")
        mm2_step(opA1, ec1, lt1, 7, 0, 512, False, True)
        mm2_step(opB, ec1, lt1, 7, 512, 896, False, True)
        mm2_step(opC, ec1, lt1, 7, 896, 1024, False, True)

        zsum1 = z_pool.tile([O, 1], F32, tag="zs")
        nc.vector.reduce_sum(zsum1, z1, axis=mybir.AxisListType.X)
        rz1 = z_pool.tile([O, 1], F32, tag="rz")
        nc.vector.reciprocal(rz1, zsum1)

        out2d1 = out[1].rearrange("(o t) -> o t", o=O)
        o_sb1 = osb_pool.tile([O, T], F32, tag="osb")
        nc.scalar.activation(
            out=o_sb1[:, 0:512], in_=opA1,
            func=mybir.ActivationFunctionType.Copy, scale=rz1,
        )
        nc.sync.dma_start(out=out2d1[:, 0:512], in_=o_sb1[:, 0:512])
        nc.vector.tensor_scalar_mul(o_sb1[:, 512:896], opB, rz1)
        nc.scalar.activation(
            out=o_sb1[:, 896:1024], in_=opC,
            func=mybir.ActivationFunctionType.Copy, scale=rz1,
        )
        nc.scalar.dma_start(out=out2d1[:, 512:1024], in_=o_sb1[:, 512:1024])

    nc.compile()
    return nc


_NC = None
TRACE = False
LAST_RESULT = None
BUILD_KWARGS = {}


def _get_nc():
    global _NC
    if _NC is None:
        _NC = build_nc(**BUILD_KWARGS)
    return _NC


def kernel(logits, decision, W, b):
    """Full-input entry point: shards batch over 8 cores, returns (16, 10240)."""
    global LAST_RESULT
    logits = np.asarray(logits, dtype=np.float32)
    W = np.asarray(W, dtype=np.float32)
    b = np.asarray(b, dtype=np.float32)
    O, D = W.shape
    P = 128
    ND = D // P
    nc = _get_nc()
    # host-side prep of tiny constants (40 KB + 64 KB + 40 B)
    wt = np.ascontiguousarray(
        W.T.reshape(ND, P, O).transpose(1, 0, 2)
    )  # [p, c, o] = W[o, 128c+p]
    b_over_o = np.ascontiguousarray((b / O).reshape(O, 1))
    ident = np.eye(P, dtype=np.float32)
    bp = B_FULL // N_CORES
    in_maps = [
        {
            "logits": np.ascontiguousarray(logits[i * bp : (i + 1) * bp]),
            "wt": wt,
            "b_over_o": b_over_o,
            "ident": ident,
        }
        for i in range(N_CORES)
    ]
    res = run_bass_kernel_spmd(nc, in_maps, core_ids=list(range(N_CORES)), trace=TRACE)
    LAST_RESULT = res
    return np.concatenate([res.results[i]["out"] for i in range(N_CORES)], axis=0)
